# revision 18
# baseline (speedup 1.0000x reference)
"""Trainium2 Bass kernel for nn_MultiHeadAttention (B=2, S=2048, E=1024, H=8, D=128).

Sharding (8 cores): core c handles batch b=c//4 and head-pair g=c%4
(heads 2g, 2g+1 -> E-columns [256g, 256g+256)).
 - Q/K/V projections column-parallel (each core computes its 256 columns).
 - Attention device-local per head, computed in transposed score layout
   scoresT[k, q] so softmaxed weights are directly the rhs of attn@V.
 - Out-projection row-parallel: each core produces a full-shape partial
   out_partial = attn_out_heads @ Wo[rows]; host sums 4 partials per batch.
 - Causal structure: fully-masked (strictly upper) 128x512 blocks are skipped;
   diagonal-straddling blocks apply the actual mask values (additively,
   pre-exp) via identity matmuls.
"""

import os
import sys

for _p in ("/opt/trn_rl_repo", os.environ.get("TRN_RL_REPO", "")):
    if _p and os.path.isdir(_p) and _p not in sys.path:
        sys.path.insert(0, _p)

import numpy as np
import ml_dtypes

BF16 = ml_dtypes.bfloat16

B, S, E, H = 2, 2048, 1024, 8
D = E // H          # 128
HP = 2              # heads per core
C = HP * D          # 256 projection columns per core
NCORES = 8
KT = S // 128       # 16 k-tiles
QC = S // 512       # 4 q-chunks
SCALE = 1.0 / float(np.sqrt(D))
MASK_NEG = -30000.0

_prog_cache = {}


def build_program(n_iters: int = 1, **opt):
    """Build the SPMD Bass program (Tile). Returns the compiled Bacc object."""
    import concourse.bass as bass
    import concourse.mybir as mybir
    import concourse.tile as tile
    from concourse import bacc, bass_isa
    from concourse.masks import make_identity
    from contextlib import ExitStack

    f32 = mybir.dt.float32
    bf16 = mybir.dt.bfloat16
    AF = mybir.ActivationFunctionType

    o = dict(xt_bufs=12, expt_bufs=2, sc_bufs=2, ot_bufs=2, proj_bufs=8,
             op_bufs=2, acc_bufs=2, part_bufs=5, outst_bufs=3,
             head_inner=False, par_red=True, all_dve=False, fuse_op=True,
             csr_f32r=False, rev_j=True, mask_mode="pmul", exp2x=True,
             out_q="sync", bias_eng="dve", xv_q="sync", red_pool=False,
             out_bf16=True, proj_n1024=False, qkbias_dve=False)
    o.update(opt)
    if o["proj_n1024"] and "proj_bufs" not in opt:
        o["proj_bufs"] = 4

    nc = bacc.Bacc("TRN2", target_bir_lowering=False, debug=False,
                   enable_partition_id=False)

    # ---- DRAM I/O (per-core slices supplied by the host) ----
    xq_t = nc.dram_tensor("xq_t", [E, S], bf16, kind="ExternalInput")
    xk_t = nc.dram_tensor("xk_t", [E, S], bf16, kind="ExternalInput")
    xv_t = nc.dram_tensor("xv_t", [E, S], bf16, kind="ExternalInput")
    wq_d = nc.dram_tensor("wq", [E, C], bf16, kind="ExternalInput")
    wk_d = nc.dram_tensor("wk", [E, C], bf16, kind="ExternalInput")
    wv_d = nc.dram_tensor("wv", [E, C], bf16, kind="ExternalInput")
    wo_d = nc.dram_tensor("wo", [C, E], bf16, kind="ExternalInput")
    bqk_d = nc.dram_tensor("bqk", [128, 4], f32, kind="ExternalInput")
    bv_d = nc.dram_tensor("bv_bc", [128, C], f32, kind="ExternalInput")
    bo_d = nc.dram_tensor("bo_bc", [128, E], f32, kind="ExternalInput")
    maskt_d = nc.dram_tensor("maskt", [KT, 128, 512], bf16, kind="ExternalInput")
    out_dt = bf16 if o["out_bf16"] else f32
    out_d = nc.dram_tensor("out", [S, E], out_dt, kind="ExternalOutput")

    with tile.TileContext(nc) as tc, ExitStack() as ctx:
        persist = ctx.enter_context(tc.tile_pool(name="persist", bufs=1))
        xt_pool = ctx.enter_context(tc.tile_pool(name="xt", bufs=o["xt_bufs"]))
        expt_pool = ctx.enter_context(tc.tile_pool(name="expt", bufs=o["expt_bufs"]))
        acc_pool = ctx.enter_context(tc.tile_pool(name="acc", bufs=o["acc_bufs"]))
        part_pool = ctx.enter_context(tc.tile_pool(name="part", bufs=o["part_bufs"]))
        outst = ctx.enter_context(tc.tile_pool(name="outst", bufs=o["outst_bufs"]))

        # ---- constants ----
        ident = persist.tile([128, 128], bf16, tag="ident")
        make_identity(nc, ident)
        f32r = mybir.dt.float32r
        ones_col = persist.tile([128, 1], f32, tag="ones_col")
        nc.vector.memset(ones_col, 1.0)
        ones_row = persist.tile([1, 128], f32, tag="ones_row")
        nc.vector.memset(ones_row, 1.0)

        # ---- persistent weight / bias / mask tiles ----
        wq_sb = persist.tile([128, 8, C], bf16, tag="wq")
        wk_sb = persist.tile([128, 8, C], bf16, tag="wk")
        wv_sb = persist.tile([128, 8, C], bf16, tag="wv")
        wo_sb = persist.tile([128, HP, E], bf16, tag="wo")
        bqk = persist.tile([128, 4], f32, tag="bqk")
        bv_bc = persist.tile([128, C], f32, tag="bv")
        bo_bc = persist.tile([128, E], f32, tag="bo")
        maskt_sb = persist.tile([128, KT, 512], bf16, tag="maskt")

        # late-needed loads go on the SWDGE queue so they don't block the
        # activation stream on the HWDGE queue
        nc.gpsimd.dma_start(out=maskt_sb,
                            in_=maskt_d.ap().rearrange("c p n -> p c n"))
        nc.gpsimd.dma_start(out=wo_sb,
                            in_=wo_d.ap().rearrange("(h p) n -> p h n", p=128))
        nc.gpsimd.dma_start(out=bo_bc, in_=bo_d.ap())
        nc.gpsimd.dma_start(out=bv_bc, in_=bv_d.ap())

        for _ in range(n_iters):
            # per-head persistent activations
            qt_sb = [persist.tile([128, S], bf16, tag=f"qt{m}", name=f"qt{m}")
                     for m in range(HP)]
            kt_sb = [persist.tile([128, S], bf16, tag=f"kt{m}", name=f"kt{m}")
                     for m in range(HP)]
            v_sb = persist.tile([128, KT, C], bf16, tag="v", name="v")
            ot_sb = [persist.tile([128, S], bf16, tag=f"ot{m}", name=f"ot{m}")
                     for m in range(HP)]

            # ================= Phase 1: projections =================
            with tc.tile_pool(name="ps_proj", bufs=o["proj_bufs"],
                              space="PSUM") as ps_proj:
                # QT / KT: [C, S] = W.T @ X.T; k-chunk outer so PE starts as
                # soon as the first 128-row chunk of X^T lands.
                for tname, xdram, wsb, qkts, bcol in (
                    ("q", xq_t, wq_sb, qt_sb, 0),
                    ("k", xk_t, wk_sb, kt_sb, 2),
                ):
                    nc.sync.dma_start(
                        out=wsb,
                        in_=(wq_d if tname == "q" else wk_d).ap()
                        .rearrange("(c p) n -> p c n", p=128))
                    if tname == "q":
                        nc.sync.dma_start(out=bqk, in_=bqk_d.ap())
                    xcs = []
                    for c in range(8):
                        xc = xt_pool.tile([128, S], bf16, tag="xtc",
                                          name=f"x{tname}{c}")
                        nc.sync.dma_start(
                            out=xc, in_=xdram[c * 128:(c + 1) * 128, :])
                        xcs.append(xc)
                    pw = 1024 if o["proj_n1024"] else 512
                    npn = S // pw
                    pss = [ps_proj.tile([128, pw], f32, tag="ps_proj",
                                        name=f"ps_{tname}{i}")
                           for i in range(HP * npn)]
                    for c in range(8):
                        for m in range(HP):
                            for n in range(npn):
                                nc.tensor.matmul(
                                    pss[m * npn + n],
                                    lhsT=wsb[:, c, m * 128:(m + 1) * 128],
                                    rhs=xcs[c][:, n * pw:(n + 1) * pw],
                                    start=(c == 0), stop=(c == 7))
                    for m in range(HP):
                        for n in range(npn):
                            if o["qkbias_dve"]:
                                nc.vector.tensor_scalar_add(
                                    qkts[m][:, n * pw:(n + 1) * pw],
                                    pss[m * npn + n],
                                    bqk[:, bcol + m:bcol + m + 1])
                            else:
                                nc.scalar.activation(
                                    out=qkts[m][:, n * pw:(n + 1) * pw],
                                    in_=pss[m * npn + n],
                                    func=AF.Identity,
                                    bias=bqk[:, bcol + m:bcol + m + 1], scale=1.0)

                # V natural: [S, C] = X @ Wv (lhsT = XT chunk slice)
                xvq = nc.sync if o["xv_q"] == "sync" else nc.scalar
                xvq.dma_start(
                    out=wv_sb, in_=wv_d.ap().rearrange("(c p) n -> p c n", p=128))
                xcs = []
                for c in range(8):
                    xc = xt_pool.tile([128, S], bf16, tag="xtc", name=f"xv{c}")
                    xvq.dma_start(out=xc, in_=xv_t[c * 128:(c + 1) * 128, :])
                    xcs.append(xc)
                for s in range(KT):
                    ps = ps_proj.tile([128, C], f32, tag="ps_proj",
                                      name=f"ps_v{s}")
                    for c in range(8):
                        nc.tensor.matmul(
                            ps,
                            lhsT=xcs[c][:, s * 128:(s + 1) * 128],
                            rhs=wv_sb[:, c, :],
                            start=(c == 0), stop=(c == 7))
                    nc.vector.tensor_add(v_sb[:, s, :], ps, bv_bc)

            # ================= Phase 2: attention (per head) =================
            with ExitStack() as p2ctx:
                ps_sc = p2ctx.enter_context(
                    tc.tile_pool(name="ps_sc", bufs=o["sc_bufs"], space="PSUM"))
                ps_ot = p2ctx.enter_context(
                    tc.tile_pool(name="ps_ot", bufs=o["ot_bufs"], space="PSUM"))
                if not o["par_red"]:
                    ps_cs = p2ctx.enter_context(
                        tc.tile_pool(name="ps_cs", bufs=1, space="PSUM"))
                    ps_rs = p2ctx.enter_context(
                        tc.tile_pool(name="ps_rs", bufs=1, space="PSUM"))
                ps_op = p2ctx.enter_context(
                    tc.tile_pool(name="ps_op", bufs=o["op_bufs"], space="PSUM"))
                jseq = (list(reversed(range(QC))) if o["rev_j"]
                        else list(range(QC)))
                hj_order = ([(h, j) for j in jseq for h in range(HP)]
                            if o["fuse_op"] else
                            [(h, j) for h in range(HP) for j in jseq])
                for h, j in hj_order:
                    nk = 4 * (j + 1)
                    ng = nk // 4
                    qsl = slice(j * 512, (j + 1) * 512)
                    et = expt_pool.tile([128, KT, 512], bf16, tag="et",
                                        name=f"et{h}{j}")
                    kstep = 2 if o["exp2x"] else 1
                    for kt0 in range(0, nk, kstep):
                        ps = ps_sc.tile([128, kstep * 512], f32, tag="ps_sc",
                                        name=f"sc{h}{j}{kt0}")
                        for u in range(kstep):
                            kti = kt0 + u
                            usl = slice(u * 512, (u + 1) * 512)
                            diag = kti >= 4 * j and o["mask_mode"] == "mm"
                            if diag:
                                nc.tensor.matmul(ps[:, usl], lhsT=ident,
                                                 rhs=maskt_sb[:, kti, :],
                                                 start=True, stop=False)
                            nc.tensor.matmul(
                                ps[:, usl],
                                lhsT=kt_sb[h][:, kti * 128:(kti + 1) * 128],
                                rhs=qt_sb[h][:, qsl],
                                start=(not diag), stop=True)
                        nc.scalar.activation(
                            out=et[:, kt0:kt0 + kstep, :].rearrange(
                                "p k n -> p (k n)"),
                            in_=ps, func=AF.Exp, scale=SCALE)
                        if o["mask_mode"] == "pmul":
                            for u in range(kstep):
                                kti = kt0 + u
                                if kti >= 4 * j:
                                    nc.gpsimd.tensor_mul(
                                        et[:, kti, :], et[:, kti, :],
                                        maskt_sb[:, kti, :])

                    # attn @ V -> outT[d, qchunk] (accumulate over k-tiles)
                    ot = ps_ot.tile([128, 512], f32, tag="ps_ot",
                                    name=f"ot{h}{j}")
                    for kti in range(nk):
                        nc.tensor.matmul(
                            ot,
                            lhsT=v_sb[:, kti, h * 128:(h + 1) * 128],
                            rhs=et[:, kti, :],
                            start=(kti == 0), stop=(kti == nk - 1))

                    # column sums: per-4-k-tile partials, then combine
                    parts = []
                    for g in range(ng):
                        pg = part_pool.tile([128, 512], f32, tag="part",
                                            name=f"pt{h}{j}{g}")
                        if (h == 0 or o["all_dve"]) and not o["red_pool"]:
                            nc.vector.tensor_reduce(
                                out=pg,
                                in_=et[:, 4 * g:4 * g + 4, :]
                                .rearrange("p k q -> p q k"),
                                axis=mybir.AxisListType.X,
                                op=mybir.AluOpType.add)
                        else:
                            a0 = part_pool.tile([128, 512], f32, tag="part",
                                                name=f"pa{h}{j}{g}")
                            nc.gpsimd.tensor_add(a0, et[:, 4 * g, :],
                                                 et[:, 4 * g + 1, :])
                            nc.gpsimd.tensor_add(pg, et[:, 4 * g + 2, :],
                                                 et[:, 4 * g + 3, :])
                            nc.gpsimd.tensor_add(pg, pg, a0)
                        parts.append(pg)
                    if ng == 1:
                        accum = parts[0]
                    else:
                        accum = acc_pool.tile([128, 512], f32, tag="accum",
                                              name=f"ac{h}{j}")
                        eng = nc.vector
                        eng.tensor_add(accum, parts[0], parts[1])
                        for g in range(2, ng):
                            eng.tensor_add(accum, accum, parts[g])

                    if o["par_red"]:
                        allred = acc_pool.tile([128, 512], f32, tag="allred",
                                               name=f"ar{h}{j}")
                        nc.gpsimd.partition_all_reduce(
                            allred, accum, channels=128,
                            reduce_op=bass_isa.ReduceOp.add)
                        rinv = acc_pool.tile([128, 512], f32, tag="rinv",
                                             name=f"ri{h}{j}")
                        nc.vector.reciprocal(rinv, allred)
                        nc.vector.tensor_mul(ot_sb[h][:, qsl], ot, rinv)
                    else:
                        # partition-reduce + broadcast via tiny PE matmuls
                        cs = ps_cs.tile([1, 512], f32, tag="ps_cs",
                                        name=f"cs{h}{j}")
                        if o["csr_f32r"]:
                            nc.tensor.matmul(cs, lhsT=ones_col.bitcast(f32r),
                                             rhs=accum.bitcast(f32r),
                                             start=True, stop=True)
                        else:
                            nc.tensor.matmul(cs, lhsT=ones_col, rhs=accum,
                                             start=True, stop=True)
                        rinv = acc_pool.tile([1, 512], f32, tag="rinv",
                                             name=f"ri{h}{j}")
                        nc.vector.reciprocal(rinv, cs)
                        rs_ps = ps_rs.tile([128, 512], f32, tag="ps_rs",
                                           name=f"rs{h}{j}")
                        if o["csr_f32r"]:
                            nc.tensor.matmul(rs_ps, lhsT=ones_row.bitcast(f32r),
                                             rhs=rinv.bitcast(f32r),
                                             start=True, stop=True)
                        else:
                            nc.tensor.matmul(rs_ps, lhsT=ones_row, rhs=rinv,
                                             start=True, stop=True)
                        rs_sb = acc_pool.tile([128, 512], f32, tag="rssb",
                                              name=f"rb{h}{j}")
                        nc.vector.tensor_copy(rs_sb, rs_ps)
                        nc.vector.tensor_mul(ot_sb[h][:, qsl], ot, rs_sb)

                    if o["fuse_op"] and h == HP - 1:
                        outq = (nc.gpsimd if o["out_q"] == "gpsimd"
                                else nc.sync if o["out_q"] == "sync"
                                else nc.scalar)
                        for s in range(4 * j, 4 * j + 4):
                            osb = outst.tile([128, E], out_dt, tag="osb",
                                             name=f"osb{s}")
                            for nch in range(2):
                                nsl = slice(nch * 512, (nch + 1) * 512)
                                ps = ps_op.tile([128, 512], f32, tag="ps_op",
                                                name=f"op{s}{nch}")
                                for hh in range(HP):
                                    nc.tensor.matmul(
                                        ps,
                                        lhsT=ot_sb[hh][:, s * 128:(s + 1) * 128],
                                        rhs=wo_sb[:, hh, nsl],
                                        start=(hh == 0), stop=(hh == HP - 1))
                                beng = (nc.vector if o["bias_eng"] == "dve"
                                        else nc.gpsimd)
                                beng.tensor_add(osb[:, nsl], ps, bo_bc[:, nsl])
                            outq.dma_start(
                                out=out_d[s * 128:(s + 1) * 128, :], in_=osb)

            # ================= Phase 3: out-projection (unfused fallback) =====
            with tc.tile_pool(name="ps_op2", bufs=o["op_bufs"],
                              space="PSUM") as ps_op2:
                for s in ([] if o["fuse_op"] else range(KT)):
                    osb = outst.tile([128, E], out_dt, tag="osb", name=f"osb{s}")
                    for nch in range(2):
                        nsl = slice(nch * 512, (nch + 1) * 512)
                        ps = ps_op2.tile([128, 512], f32, tag="ps_op",
                                        name=f"op{s}{nch}")
                        for h in range(HP):
                            nc.tensor.matmul(
                                ps,
                                lhsT=ot_sb[h][:, s * 128:(s + 1) * 128],
                                rhs=wo_sb[:, h, nsl],
                                start=(h == 0), stop=(h == HP - 1))
                        nc.vector.tensor_add(osb[:, nsl], ps, bo_bc[:, nsl])
                    nc.gpsimd.dma_start(out=out_d[s * 128:(s + 1) * 128, :],
                                        in_=osb)

    nc.compile()
    return nc


def get_program(n_iters: int = 1):
    if n_iters not in _prog_cache:
        _prog_cache[n_iters] = build_program(n_iters)
    return _prog_cache[n_iters]


def make_in_maps(query, key_, value, Wq, bq, Wk, bk, Wv, bv, Wo, bo, mask,
                 mask_mode="pmul"):
    """Host-side sharding: build the 8 per-core input maps."""
    query = np.asarray(query, np.float32)
    key_ = np.asarray(key_, np.float32)
    value = np.asarray(value, np.float32)
    mask = np.asarray(mask)

    # transposed bf16 activations per batch: [E, S]
    xt = {}
    for b in range(B):
        xt[("q", b)] = np.ascontiguousarray(query[b].T.astype(BF16))
        xt[("k", b)] = np.ascontiguousarray(key_[b].T.astype(BF16))
        xt[("v", b)] = np.ascontiguousarray(value[b].T.astype(BF16))

    # transposed mask, diagonal 128x512 blocks only.
    # mask_mode "mm": additive pre-scale values; "pmul": 0/1 multiplicative.
    m2 = np.asarray(mask).reshape(S, S)
    maskt = np.empty((KT, 128, 512), np.float32)
    for j in range(QC):
        q0 = j * 512
        blk = m2[q0:q0 + 512, q0:q0 + 512]           # [q, k]
        if mask_mode == "pmul":
            add = np.where(blk.T != 0, 1.0, 0.0)     # [k, q]
        else:
            add = np.where(blk.T != 0, 0.0, MASK_NEG) / SCALE
        for i in range(4):
            maskt[4 * j + i] = add[i * 128:(i + 1) * 128, :]
    maskt = maskt.astype(BF16)

    Wq = np.asarray(Wq, np.float32)
    Wk = np.asarray(Wk, np.float32)
    Wv = np.asarray(Wv, np.float32)
    Wo = np.asarray(Wo, np.float32)
    bq = np.asarray(bq, np.float32)
    bk = np.asarray(bk, np.float32)
    bv = np.asarray(bv, np.float32)
    bo = np.asarray(bo, np.float32)

    in_maps = []
    for c in range(NCORES):
        b, g = divmod(c, 4)
        c0 = C * g
        bqk = np.stack([bq[c0:c0 + 128], bq[c0 + 128:c0 + 256],
                        bk[c0:c0 + 128], bk[c0 + 128:c0 + 256]], axis=1)
        in_maps.append({
            "xq_t": xt[("q", b)],
            "xk_t": xt[("k", b)],
            "xv_t": xt[("v", b)],
            "wq": Wq[:, c0:c0 + C].astype(BF16),
            "wk": Wk[:, c0:c0 + C].astype(BF16),
            "wv": Wv[:, c0:c0 + C].astype(BF16),
            "wo": np.ascontiguousarray(Wo[c0:c0 + C, :]).astype(BF16),
            "bqk": np.ascontiguousarray(bqk, dtype=np.float32),
            "bv_bc": np.broadcast_to(bv[c0:c0 + C], (128, C)).astype(np.float32),
            "bo_bc": (np.broadcast_to(bo, (128, E)).astype(np.float32)
                      if g == 0 else np.zeros((128, E), np.float32)),
            "maskt": maskt,
        })
    return in_maps


def gather_output(results):
    out = np.zeros((B, S, E), np.float32)
    for c in range(NCORES):
        b = c // 4
        out[b] += results[c]["out"]
    return out


def kernel(**inputs) -> np.ndarray:
    from concourse.bass_utils import run_bass_kernel_spmd

    nc = get_program(1)
    in_maps = make_in_maps(**inputs)
    res = run_bass_kernel_spmd(nc, in_maps, core_ids=list(range(NCORES)))
    return gather_output(res.results)



# revision 43
# speedup vs baseline: 1.9475x; 1.9475x over previous
"""Trainium2 Bass kernel for nn_MultiHeadAttention (B=2, S=2048, E=1024, H=8, D=128).

Sharding (8 cores): core c handles batch b=c//4 and head-pair g=c%4
(heads 2g, 2g+1 -> E-columns [256g, 256g+256)).
 - Q/K/V projections column-parallel (each core computes its 256 columns).
 - Attention device-local per head, computed in transposed score layout
   scoresT[k, q] so softmaxed weights are directly the rhs of attn@V.
 - Out-projection row-parallel: each core produces a full-shape partial
   out_partial = attn_out_heads @ Wo[rows]; host sums 4 partials per batch.
 - Causal structure: fully-masked (strictly upper) 128x512 blocks are skipped;
   diagonal-straddling blocks apply the actual mask values (additively,
   pre-exp) via identity matmuls.
"""

import os
import sys

for _p in ("/opt/trn_rl_repo", os.environ.get("TRN_RL_REPO", "")):
    if _p and os.path.isdir(_p) and _p not in sys.path:
        sys.path.insert(0, _p)

import numpy as np
import ml_dtypes

BF16 = ml_dtypes.bfloat16

B, S, E, H = 2, 2048, 1024, 8
D = E // H          # 128
HP = 2              # heads per core
C = HP * D          # 256 projection columns per core
NCORES = 8
KT = S // 128       # 16 k-tiles
QC = S // 512       # 4 q-chunks
SCALE = 1.0 / float(np.sqrt(D))
MASK_NEG = -30000.0

_prog_cache = {}


# Default build configuration — kernel() and make_in_maps() must agree on
# mask_mode / out_bf16 / fp8_proj, so both read from this dict.
DEFAULT_CFG = dict(
    xt_bufs=12, expt_bufs=2, sc_bufs=2, ot_bufs=2, proj_bufs=8,
    op_bufs=2, acc_bufs=2, part_bufs=5, outst_bufs=3,
    head_inner=False, par_red=False, all_dve=False, fuse_op=True,
    csr_f32r=False, rev_j=True, mask_mode="pmul", exp2x=False,
    out_q="gpsimd", bias_eng="dve", xv_q="sync", red_pool=False,
    out_bf16=True, proj_n1024=False, qkbias_dve=True,
    fp8_proj=False, x8_bufs=4, den_mm=True, proj_waves=1, den_split=False)


def build_program(n_iters: int = 1, **opt):
    """Build the SPMD Bass program (Tile). Returns the compiled Bacc object."""
    import concourse.bass as bass
    import concourse.mybir as mybir
    import concourse.tile as tile
    from concourse import bacc, bass_isa
    from concourse.masks import make_identity
    from contextlib import ExitStack

    f32 = mybir.dt.float32
    bf16 = mybir.dt.bfloat16
    AF = mybir.ActivationFunctionType

    o = dict(DEFAULT_CFG)
    o.update(opt)
    if o["proj_n1024"] and "proj_bufs" not in opt:
        o["proj_bufs"] = 4
    if o["proj_waves"] == 2 and "proj_bufs" not in opt:
        o["proj_bufs"] = 4

    nc = bacc.Bacc("TRN2", target_bir_lowering=False, debug=False,
                   enable_partition_id=False)
    if o["csr_f32r"]:
        ctx_lp = nc.allow_low_precision(reason="f32r rowsum matmuls")
        ctx_lp.__enter__()

    # ---- DRAM I/O (per-core slices supplied by the host) ----
    fp8 = mybir.dt.float8e4
    xw_dt = fp8 if o["fp8_proj"] else bf16
    PM = (mybir.MatmulPerfMode.DoubleRow if o["fp8_proj"] else None)
    WSI = (1.0 / 32.0) if o["fp8_proj"] else 1.0
    xq_t = nc.dram_tensor("xq_t", [E, S], xw_dt, kind="ExternalInput")
    xk_t = nc.dram_tensor("xk_t", [E, S], xw_dt, kind="ExternalInput")
    xv_t = nc.dram_tensor("xv_t", [E, S], xw_dt, kind="ExternalInput")
    wq_d = nc.dram_tensor("wq", [E, C], xw_dt, kind="ExternalInput")
    wk_d = nc.dram_tensor("wk", [E, C], xw_dt, kind="ExternalInput")
    wv_d = nc.dram_tensor("wv", [E, C], xw_dt, kind="ExternalInput")
    wo_d = nc.dram_tensor("wo", [C, E], bf16, kind="ExternalInput")
    bqk_d = nc.dram_tensor("bqk", [128, 4], f32, kind="ExternalInput")
    bv_d = nc.dram_tensor("bv_bc", [128, C], f32, kind="ExternalInput")
    bo_d = nc.dram_tensor("bo_bc", [128, E], f32, kind="ExternalInput")
    maskt_d = nc.dram_tensor("maskt", [KT, 128, 512], bf16, kind="ExternalInput")
    out_dt = bf16 if o["out_bf16"] else f32
    out_d = nc.dram_tensor("out", [S, E], out_dt, kind="ExternalOutput")

    with tile.TileContext(nc) as tc, ExitStack() as ctx:
        persist = ctx.enter_context(tc.tile_pool(name="persist", bufs=1))
        xt_pool = ctx.enter_context(tc.tile_pool(
            name="xt", bufs=o["x8_bufs"] if o["fp8_proj"] else o["xt_bufs"]))
        expt_pool = ctx.enter_context(tc.tile_pool(name="expt", bufs=o["expt_bufs"]))
        acc_pool = ctx.enter_context(tc.tile_pool(name="acc", bufs=o["acc_bufs"]))
        part_pool = ctx.enter_context(tc.tile_pool(name="part", bufs=o["part_bufs"]))
        outst = ctx.enter_context(tc.tile_pool(name="outst", bufs=o["outst_bufs"]))

        # ---- constants ----
        ident = persist.tile([128, 128], bf16, tag="ident")
        make_identity(nc, ident)
        f32r = mybir.dt.float32r
        acc_dt = f32r if o["csr_f32r"] else f32
        ones_col = persist.tile([128, 1], acc_dt, tag="ones_col")
        nc.vector.memset(ones_col, 1.0)
        ones_row = persist.tile([1, 128], acc_dt, tag="ones_row")
        nc.vector.memset(ones_row, 1.0)
        if o["den_mm"]:
            ones128 = persist.tile([128, 128], bf16, tag="ones128")
            nc.vector.memset(ones128, 1.0)

        # ---- persistent weight / bias / mask tiles ----
        wq_sb = persist.tile([128, 8, C], xw_dt, tag="wq")
        wk_sb = persist.tile([128, 8, C], xw_dt, tag="wk")
        wv_sb = persist.tile([128, 8, C], xw_dt, tag="wv")
        wo_sb = persist.tile([128, HP, E], bf16, tag="wo")
        bqk = persist.tile([128, 4], f32, tag="bqk")
        bv_bc = persist.tile([128, C], f32, tag="bv")
        bo_bc = persist.tile([128, E], f32, tag="bo")
        maskt_sb = persist.tile([128, KT, 512], bf16, tag="maskt")

        # late-needed loads go on the SWDGE queue so they don't block the
        # activation stream on the HWDGE queue
        nc.gpsimd.dma_start(out=maskt_sb,
                            in_=maskt_d.ap().rearrange("c p n -> p c n"))
        nc.gpsimd.dma_start(out=wo_sb,
                            in_=wo_d.ap().rearrange("(h p) n -> p h n", p=128))
        nc.gpsimd.dma_start(out=bo_bc, in_=bo_d.ap())
        nc.gpsimd.dma_start(out=bv_bc, in_=bv_d.ap())

        for _ in range(n_iters):
            # per-head persistent activations
            qt_sb = [persist.tile([128, S], bf16, tag=f"qt{m}", name=f"qt{m}")
                     for m in range(HP)]
            kt_sb = [persist.tile([128, S], bf16, tag=f"kt{m}", name=f"kt{m}")
                     for m in range(HP)]
            v_sb = persist.tile([128, KT, C], bf16, tag="v", name="v")
            ot_sb = [persist.tile([128, S], bf16, tag=f"ot{m}", name=f"ot{m}")
                     for m in range(HP)]

            # ================= Phase 1: projections =================
            with tc.tile_pool(name="ps_proj", bufs=o["proj_bufs"],
                              space="PSUM") as ps_proj:
                # QT / KT: [C, S] = W.T @ X.T; k-chunk outer so PE starts as
                # soon as the first 128-row chunk of X^T lands.
                # fp8 mode: x held as one [128, 8, S] tile; chunk-pair
                # DoubleRow matmuls contract 256 rows at a time.
                for tname, xdram, wsb, qkts, bcol in (
                    ("q", xq_t, wq_sb, qt_sb, 0),
                    ("k", xk_t, wk_sb, kt_sb, 2),
                ):
                    nc.sync.dma_start(
                        out=wsb,
                        in_=(wq_d if tname == "q" else wk_d).ap()
                        .rearrange("(c p) n -> p c n", p=128))
                    if tname == "q":
                        nc.sync.dma_start(out=bqk, in_=bqk_d.ap())
                    if o["fp8_proj"]:
                        x8 = xt_pool.tile([128, 8, S], fp8, tag="x8",
                                          name=f"x8{tname}")
                        for c in range(8):
                            nc.sync.dma_start(
                                out=x8[:, c, :],
                                in_=xdram[c * 128:(c + 1) * 128, :])
                    else:
                        xcs = []
                        for c in range(8):
                            xc = xt_pool.tile([128, S], bf16, tag="xtc",
                                              name=f"x{tname}{c}")
                            nc.sync.dma_start(
                                out=xc, in_=xdram[c * 128:(c + 1) * 128, :])
                            xcs.append(xc)
                    pw = 1024 if o["proj_n1024"] else 512
                    npn = S // pw
                    nwav = o["proj_waves"]
                    for m0 in range(0, HP, HP // nwav):
                        mset = range(m0, m0 + HP // nwav)
                        pss = {(m, n): ps_proj.tile(
                                   [128, pw], f32, tag="ps_proj",
                                   name=f"ps_{tname}{m}_{n}")
                               for m in mset for n in range(npn)}
                        for c in range(4 if o["fp8_proj"] else 8):
                            for m in mset:
                                for n in range(npn):
                                    if o["fp8_proj"]:
                                        nc.tensor.matmul(
                                            pss[m, n],
                                            lhsT=wsb[:, 2 * c:2 * c + 2,
                                                     m * 128:(m + 1) * 128],
                                            rhs=x8[:, 2 * c:2 * c + 2,
                                                   n * pw:(n + 1) * pw],
                                            start=(c == 0), stop=(c == 3),
                                            perf_mode=PM)
                                    else:
                                        nc.tensor.matmul(
                                            pss[m, n],
                                            lhsT=wsb[:, c,
                                                     m * 128:(m + 1) * 128],
                                            rhs=xcs[c][:, n * pw:(n + 1) * pw],
                                            start=(c == 0), stop=(c == 7))
                        for m in mset:
                            for n in range(npn):
                                if o["qkbias_dve"] and o["fp8_proj"]:
                                    nc.vector.tensor_scalar(
                                        qkts[m][:, n * pw:(n + 1) * pw],
                                        pss[m, n],
                                        WSI, bqk[:, bcol + m:bcol + m + 1],
                                        op0=mybir.AluOpType.mult,
                                        op1=mybir.AluOpType.add)
                                elif o["qkbias_dve"]:
                                    nc.vector.tensor_scalar_add(
                                        qkts[m][:, n * pw:(n + 1) * pw],
                                        pss[m, n],
                                        bqk[:, bcol + m:bcol + m + 1])
                                else:
                                    nc.scalar.activation(
                                        out=qkts[m][:, n * pw:(n + 1) * pw],
                                        in_=pss[m, n],
                                        func=AF.Identity,
                                        bias=bqk[:, bcol + m:bcol + m + 1],
                                        scale=WSI)

                # V natural: [S, C] = X @ Wv (lhsT = XT chunk slice)
                xvq = nc.sync if o["xv_q"] == "sync" else nc.scalar
                xvq.dma_start(
                    out=wv_sb, in_=wv_d.ap().rearrange("(c p) n -> p c n", p=128))
                if o["fp8_proj"]:
                    x8v = xt_pool.tile([128, 8, S], fp8, tag="x8", name="x8v")
                    for c in range(8):
                        xvq.dma_start(out=x8v[:, c, :],
                                      in_=xv_t[c * 128:(c + 1) * 128, :])
                else:
                    xcs = []
                    for c in range(8):
                        xc = xt_pool.tile([128, S], bf16, tag="xtc",
                                          name=f"xv{c}")
                        xvq.dma_start(out=xc,
                                      in_=xv_t[c * 128:(c + 1) * 128, :])
                        xcs.append(xc)
                for s in range(KT):
                    ps = ps_proj.tile([128, C], f32, tag="ps_proj",
                                      name=f"ps_v{s}")
                    if o["fp8_proj"]:
                        for c in range(4):
                            nc.tensor.matmul(
                                ps,
                                lhsT=x8v[:, 2 * c:2 * c + 2,
                                         s * 128:(s + 1) * 128],
                                rhs=wv_sb[:, 2 * c:2 * c + 2, :],
                                start=(c == 0), stop=(c == 3),
                                perf_mode=PM)
                    else:
                        for c in range(8):
                            nc.tensor.matmul(
                                ps,
                                lhsT=xcs[c][:, s * 128:(s + 1) * 128],
                                rhs=wv_sb[:, c, :],
                                start=(c == 0), stop=(c == 7))
                    nc.vector.tensor_add(v_sb[:, s, :], ps, bv_bc)

            # ================= Phase 2: attention (per head) =================
            with ExitStack() as p2ctx:
                ps_sc = p2ctx.enter_context(
                    tc.tile_pool(name="ps_sc", bufs=o["sc_bufs"], space="PSUM"))
                ps_ot = p2ctx.enter_context(
                    tc.tile_pool(name="ps_ot", bufs=o["ot_bufs"], space="PSUM"))
                if o["den_mm"]:
                    ps_den = p2ctx.enter_context(
                        tc.tile_pool(name="ps_den", bufs=2, space="PSUM"))
                elif not o["par_red"]:
                    ps_cs = p2ctx.enter_context(
                        tc.tile_pool(name="ps_cs", bufs=1, space="PSUM"))
                    ps_rs = p2ctx.enter_context(
                        tc.tile_pool(name="ps_rs", bufs=1, space="PSUM"))
                ps_op = p2ctx.enter_context(
                    tc.tile_pool(name="ps_op", bufs=o["op_bufs"], space="PSUM"))
                jseq = (list(reversed(range(QC))) if o["rev_j"]
                        else list(range(QC)))
                hj_order = ([(h, j) for j in jseq for h in range(HP)]
                            if o["fuse_op"] else
                            [(h, j) for h in range(HP) for j in jseq])

                def emit_out_proj(j):
                    outq = (nc.gpsimd if o["out_q"] == "gpsimd"
                            else nc.sync if o["out_q"] == "sync"
                            else nc.scalar)
                    for s in range(4 * j, 4 * j + 4):
                        osb = outst.tile([128, E], out_dt, tag="osb",
                                         name=f"osb{s}")
                        # hh outer so both nch matmuls reuse the stationary
                        # operand (LDWEIGHTS dedup on consecutive same-lhsT)
                        pss2 = [ps_op.tile([128, 512], f32, tag="ps_op",
                                           name=f"op{s}{nch}")
                                for nch in range(2)]
                        for hh in range(HP):
                            for nch in range(2):
                                nsl = slice(nch * 512, (nch + 1) * 512)
                                nc.tensor.matmul(
                                    pss2[nch],
                                    lhsT=ot_sb[hh][:, s * 128:(s + 1) * 128],
                                    rhs=wo_sb[:, hh, nsl],
                                    start=(hh == 0), stop=(hh == HP - 1))
                        beng = (nc.vector if o["bias_eng"] == "dve"
                                else nc.gpsimd)
                        for nch in range(2):
                            nsl = slice(nch * 512, (nch + 1) * 512)
                            beng.tensor_add(osb[:, nsl], pss2[nch],
                                            bo_bc[:, nsl])
                        outq.dma_start(
                            out=out_d[s * 128:(s + 1) * 128, :], in_=osb)

                pending_op = [None]

                for h, j in hj_order:
                    nk = 4 * (j + 1)
                    ng = nk // 4
                    qsl = slice(j * 512, (j + 1) * 512)
                    et = expt_pool.tile([128, KT, 512], bf16, tag="et",
                                        name=f"et{h}{j}")
                    kstep = 2 if o["exp2x"] else 1
                    for kt0 in range(0, nk, kstep):
                        ps = ps_sc.tile([128, kstep * 512], f32, tag="ps_sc",
                                        name=f"sc{h}{j}{kt0}")
                        for u in range(kstep):
                            kti = kt0 + u
                            usl = slice(u * 512, (u + 1) * 512)
                            diag = kti >= 4 * j and o["mask_mode"] == "mm"
                            if diag:
                                nc.tensor.matmul(ps[:, usl], lhsT=ident,
                                                 rhs=maskt_sb[:, kti, :],
                                                 start=True, stop=False)
                            nc.tensor.matmul(
                                ps[:, usl],
                                lhsT=kt_sb[h][:, kti * 128:(kti + 1) * 128],
                                rhs=qt_sb[h][:, qsl],
                                start=(not diag), stop=True)
                        nc.scalar.activation(
                            out=et[:, kt0:kt0 + kstep, :].rearrange(
                                "p k n -> p (k n)"),
                            in_=ps, func=AF.Exp, scale=SCALE)
                        if o["mask_mode"] == "pmul":
                            for u in range(kstep):
                                kti = kt0 + u
                                if kti >= 4 * j:
                                    nc.gpsimd.tensor_mul(
                                        et[:, kti, :], et[:, kti, :],
                                        maskt_sb[:, kti, :])

                    # deferred out-proj of the previous q-chunk: emitted
                    # after this group's scores so the in-order PE queue has
                    # score work while out-proj waits on the DVE mul
                    if pending_op[0] is not None:
                        pending_op[0]()
                        pending_op[0] = None

                    # attn @ V -> outT[d, qchunk] (accumulate over k-tiles)
                    ot = ps_ot.tile([128, 512], f32, tag="ps_ot",
                                    name=f"ot{h}{j}")
                    for kti in range(nk):
                        nc.tensor.matmul(
                            ot,
                            lhsT=v_sb[:, kti, h * 128:(h + 1) * 128],
                            rhs=et[:, kti, :],
                            start=(kti == 0), stop=(kti == nk - 1))

                    if o["den_mm"]:
                        # rowsums via all-ones stationary: every output
                        # partition gets the partition-sum (pre-broadcast)
                        den = ps_den.tile([128, 512], f32, tag="ps_den",
                                          name=f"dn{h}{j}")
                        for kti in range(nk):
                            nc.tensor.matmul(
                                den, lhsT=ones128, rhs=et[:, kti, :],
                                start=(kti == 0), stop=(kti == nk - 1))
                        rinv = acc_pool.tile([128, 512], f32, tag="rinv",
                                             name=f"ri{h}{j}")
                        nc.vector.reciprocal(rinv, den)
                        nc.vector.tensor_mul(ot_sb[h][:, qsl], ot, rinv)
                        if o["fuse_op"] and h == HP - 1:
                            pending_op[0] = (lambda jj=j: emit_out_proj(jj))
                        continue

                    # column sums: per-4-k-tile partials, then combine
                    parts = []
                    part_dt = acc_dt if ng == 1 else f32
                    for g in range(ng):
                        pg = part_pool.tile([128, 512], part_dt, tag="part",
                                            name=f"pt{h}{j}{g}")
                        if (h == 0 or o["all_dve"]) and not o["red_pool"]:
                            nc.vector.tensor_reduce(
                                out=pg,
                                in_=et[:, 4 * g:4 * g + 4, :]
                                .rearrange("p k q -> p q k"),
                                axis=mybir.AxisListType.X,
                                op=mybir.AluOpType.add)
                        else:
                            a0 = part_pool.tile([128, 512], f32, tag="part",
                                                name=f"pa{h}{j}{g}")
                            nc.gpsimd.tensor_add(a0, et[:, 4 * g, :],
                                                 et[:, 4 * g + 1, :])
                            nc.gpsimd.tensor_add(pg, et[:, 4 * g + 2, :],
                                                 et[:, 4 * g + 3, :])
                            nc.gpsimd.tensor_add(pg, pg, a0)
                        parts.append(pg)
                    if ng == 1:
                        accum = parts[0]
                    else:
                        accum = acc_pool.tile([128, 512], acc_dt, tag="accum",
                                              name=f"ac{h}{j}")
                        eng = nc.vector
                        eng.tensor_add(accum, parts[0], parts[1])
                        for g in range(2, ng):
                            eng.tensor_add(accum, accum, parts[g])

                    if o["par_red"]:
                        allred = acc_pool.tile([128, 512], f32, tag="allred",
                                               name=f"ar{h}{j}")
                        nc.gpsimd.partition_all_reduce(
                            allred, accum, channels=128,
                            reduce_op=bass_isa.ReduceOp.add)
                        rinv = acc_pool.tile([128, 512], f32, tag="rinv",
                                             name=f"ri{h}{j}")
                        nc.vector.reciprocal(rinv, allred)
                        nc.vector.tensor_mul(ot_sb[h][:, qsl], ot, rinv)
                    else:
                        # partition-reduce + broadcast via tiny PE matmuls
                        cs = ps_cs.tile([1, 512], f32, tag="ps_cs",
                                        name=f"cs{h}{j}")
                        nc.tensor.matmul(cs, lhsT=ones_col, rhs=accum,
                                         start=True, stop=True)
                        rinv = acc_pool.tile([1, 512], acc_dt, tag="rinv",
                                             name=f"ri{h}{j}")
                        nc.vector.reciprocal(rinv, cs)
                        rs_ps = ps_rs.tile([128, 512], f32, tag="ps_rs",
                                           name=f"rs{h}{j}")
                        nc.tensor.matmul(rs_ps, lhsT=ones_row, rhs=rinv,
                                         start=True, stop=True)
                        rs_sb = acc_pool.tile([128, 512], f32, tag="rssb",
                                              name=f"rb{h}{j}")
                        nc.vector.tensor_copy(rs_sb, rs_ps)
                        nc.vector.tensor_mul(ot_sb[h][:, qsl], ot, rs_sb)

                    if o["fuse_op"] and h == HP - 1:
                        pending_op[0] = (lambda jj=j: emit_out_proj(jj))

                if pending_op[0] is not None:
                    pending_op[0]()
                    pending_op[0] = None

            # ================= Phase 3: out-projection (unfused fallback) =====
            with tc.tile_pool(name="ps_op2", bufs=o["op_bufs"],
                              space="PSUM") as ps_op2:
                for s in ([] if o["fuse_op"] else range(KT)):
                    osb = outst.tile([128, E], out_dt, tag="osb", name=f"osb{s}")
                    for nch in range(2):
                        nsl = slice(nch * 512, (nch + 1) * 512)
                        ps = ps_op2.tile([128, 512], f32, tag="ps_op",
                                        name=f"op{s}{nch}")
                        for h in range(HP):
                            nc.tensor.matmul(
                                ps,
                                lhsT=ot_sb[h][:, s * 128:(s + 1) * 128],
                                rhs=wo_sb[:, h, nsl],
                                start=(h == 0), stop=(h == HP - 1))
                        nc.vector.tensor_add(osb[:, nsl], ps, bo_bc[:, nsl])
                    nc.gpsimd.dma_start(out=out_d[s * 128:(s + 1) * 128, :],
                                        in_=osb)

    nc.compile()
    return nc


def get_program(n_iters: int = 1, **opt):
    key = (n_iters, tuple(sorted(opt.items())))
    if key not in _prog_cache:
        _prog_cache[key] = build_program(n_iters, **opt)
    return _prog_cache[key]


def make_in_maps(query, key_, value, Wq, bq, Wk, bk, Wv, bv, Wo, bo, mask,
                 mask_mode=None, fp8_proj=None):
    """Host-side sharding: build the 8 per-core input maps."""
    if mask_mode is None:
        mask_mode = DEFAULT_CFG["mask_mode"]
    if fp8_proj is None:
        fp8_proj = DEFAULT_CFG["fp8_proj"]
    FP8 = ml_dtypes.float8_e4m3
    x_dt = FP8 if fp8_proj else BF16
    ws = 32.0 if fp8_proj else 1.0  # fp8 weight pre-scale (denormal escape)
    query = np.asarray(query, np.float32)
    key_ = np.asarray(key_, np.float32)
    value = np.asarray(value, np.float32)
    mask = np.asarray(mask)

    # transposed activations per batch: [E, S]
    xt = {}
    for b in range(B):
        xt[("q", b)] = np.ascontiguousarray(query[b].T).astype(x_dt)
        xt[("k", b)] = np.ascontiguousarray(key_[b].T).astype(x_dt)
        xt[("v", b)] = np.ascontiguousarray(value[b].T).astype(x_dt)

    # transposed mask, diagonal 128x512 blocks only.
    # mask_mode "mm": additive pre-scale values; "pmul": 0/1 multiplicative.
    m2 = np.asarray(mask).reshape(S, S)
    maskt = np.empty((KT, 128, 512), np.float32)
    for j in range(QC):
        q0 = j * 512
        blk = m2[q0:q0 + 512, q0:q0 + 512]           # [q, k]
        if mask_mode == "pmul":
            add = np.where(blk.T != 0, 1.0, 0.0)     # [k, q]
        else:
            add = np.where(blk.T != 0, 0.0, MASK_NEG) / SCALE
        for i in range(4):
            maskt[4 * j + i] = add[i * 128:(i + 1) * 128, :]
    maskt = maskt.astype(BF16)

    Wq = np.asarray(Wq, np.float32)
    Wk = np.asarray(Wk, np.float32)
    Wv = np.asarray(Wv, np.float32)
    Wo = np.asarray(Wo, np.float32)
    bq = np.asarray(bq, np.float32)
    bk = np.asarray(bk, np.float32)
    bv = np.asarray(bv, np.float32)
    bo = np.asarray(bo, np.float32)

    in_maps = []
    for c in range(NCORES):
        b, g = divmod(c, 4)
        c0 = C * g
        bqk = np.stack([bq[c0:c0 + 128], bq[c0 + 128:c0 + 256],
                        bk[c0:c0 + 128], bk[c0 + 128:c0 + 256]], axis=1)
        in_maps.append({
            "xq_t": xt[("q", b)],
            "xk_t": xt[("k", b)],
            "xv_t": xt[("v", b)],
            "wq": (ws * Wq[:, c0:c0 + C]).astype(x_dt),
            "wk": (ws * Wk[:, c0:c0 + C]).astype(x_dt),
            "wv": (ws * Wv[:, c0:c0 + C]).astype(x_dt),
            "wo": np.ascontiguousarray(Wo[c0:c0 + C, :] / ws).astype(BF16),
            "bqk": np.ascontiguousarray(bqk, dtype=np.float32),
            "bv_bc": (ws * np.broadcast_to(bv[c0:c0 + C], (128, C))).astype(np.float32),
            "bo_bc": (np.broadcast_to(bo, (128, E)).astype(np.float32)
                      if g == 0 else np.zeros((128, E), np.float32)),
            "maskt": maskt,
        })
    return in_maps


def gather_output(results):
    out = np.zeros((B, S, E), np.float32)
    for c in range(NCORES):
        b = c // 4
        out[b] += results[c]["out"]
    return out


def kernel(**inputs) -> np.ndarray:
    from concourse.bass_utils import run_bass_kernel_spmd

    nc = get_program(1)
    in_maps = make_in_maps(**inputs)
    res = run_bass_kernel_spmd(nc, in_maps, core_ids=list(range(NCORES)))
    return gather_output(res.results)

